# revision 1
# baseline (speedup 1.0000x reference)
"""Cosine-similarity self-attention (Cos_Attn) on 8 Trainium2 NeuronCores.

Reference math (x: [C=512, W=64, H=64] fp32, N = W*H = 4096):
    q = x.reshape(C, N).T                  # [N, C]
    energy = q @ q.T                       # [N, N]
    cos    = energy / (|q_i| |q_j|)
    out    = softmax(cos, axis=-1)[None]   # [1, N, N]

Sharding: the N query rows are split across 8 cores (512 rows each). Every
core receives the full x (the keys) plus its own query column slice
xq = x[:, rows]; it computes its [512, N] slice of cos and the row softmax
locally; the host concatenates the 8 slices.

Per-core device pipeline (streamed in 512-column blocks):
  -  input x arrives in per-block DMAs spread over the three DMA-capable
     issue engines (SP + ACT hardware-DGE queues, GpSimd software-DGE) -
     a single queue measured only ~70-105 GB/s and paced the whole kernel.
  -  norms: squares (GPSIMD/DVE) -> ones-matmul column-sum (PE, bf16) into
     a 4-bank PSUM strip; rn = exp(-0.5*ln(ns)) on ACT in two 2048-wide
     chunks (Ln/Exp table-set switches cost ~1.3us each, so few wide chunks
     beat per-block ones).
  -  xn = x * rn into bf16 tiles (DVE; ko-broadcast of rn, a pattern
     validated on HW) - bf16 operands give full-rate PE + fast weight load.
  -  energy tiles = xnq^T @ xn (PE, bf16), K=4x128 accumulated in PSUM;
     softmax exp straight out of PSUM on ACT with accum_out row sums
     (max-subtraction skipped: cos is bounded in [-1, 1]).
  -  row scale 1/rowsum: per-partition scale on ACT (architectural
     free-affine operand) for half the row tiles, DVE multiply by a
     materialized [P,512] scale row (middle-dim stride-0 broadcast) for the
     other half; innermost-stride-0 APs and pointer-scalar TENSOR_SCALAR
     are avoided (measured wrong / 10x slow on HW).
"""

import numpy as np

_NCORES = 8
_P = 128

# set by the test harness only; the grading path keeps these defaults
TRACE = False
TRACE_CORES = None
LAST_RESULT = None

_built = None  # (nc, C, N)


def _build(C, N, RPC):
    """Build the single-NEFF Bass/Tile program.

    Inputs:  x [C, N] (all keys), xq [C, RPC] (this core's query columns).
    Output:  out [RPC, N] = softmax rows for this core's queries.
    """
    from contextlib import ExitStack

    import concourse.tile as tile
    from concourse import bacc, mybir

    f32 = mybir.dt.float32
    bf16 = mybir.dt.bfloat16
    AF = mybir.ActivationFunctionType
    AX = mybir.AxisListType
    OP = mybir.AluOpType

    P = _P
    KO = C // P              # contraction subtiles
    CB = 512                 # column block: one PSUM bank per energy tile
    NB = N // CB
    MT = RPC // P            # query row tiles per core
    HALF = max(KO // 2, 1)
    NH = KO // HALF          # square half-chunks per block
    RNC = min(NB, 4)         # rn chunk = RNC blocks (2048 cols)
    NRN = NB // RNC

    nc = bacc.Bacc("TRN2", target_bir_lowering=False, debug=False)
    x_d = nc.dram_tensor("x", [C, N], f32, kind="ExternalInput")
    xq_d = nc.dram_tensor("xq", [C, RPC], f32, kind="ExternalInput")
    out_d = nc.dram_tensor("out", [RPC, N], f32, kind="ExternalOutput")

    x_r = x_d.ap().rearrange("(ko p) n -> p ko n", p=P)
    xq_r = xq_d.ap().rearrange("(ko p) m -> p ko m", p=P)
    out_r = out_d.ap().rearrange("(mo p) n -> p mo n", p=P)

    with tile.TileContext(nc) as tc, ExitStack() as ctx:
        persist = ctx.enter_context(tc.tile_pool(name="persist", bufs=1))
        temps = ctx.enter_context(tc.tile_pool(name="temps", bufs=3))
        psum = ctx.enter_context(tc.tile_pool(name="psum", bufs=4, space="PSUM"))

        xn_sb = persist.tile([P, KO, N], bf16)     # normalized keys
        xnq_sb = persist.tile([P, KO, RPC], bf16)  # normalized queries
        rn = persist.tile([P, N], f32)             # 1/|q_n|, replicated on parts
        rnq = persist.tile([P, RPC], f32)
        e = persist.tile([P, MT, N], f32)          # exp(cos); scaled in place
        sums = persist.tile([P, MT, NB], f32)      # per-(m, nb) exp row sums
        rs = persist.tile([P, MT], f32)
        rr = persist.tile([P, MT], f32)
        ones = persist.tile([P, P], bf16)
        ones_row = persist.tile([P, CB], f32)
        nc.vector.memset(ones[:], 1.0)
        nc.vector.memset(ones_row[:], 1.0)

        # round-robin DMA issue over the three DMA-capable engines so the
        # transfers spread across independent DGE queues
        dma_engines = [nc.sync, nc.scalar, nc.gpsimd]
        dma_state = [0]

        def dma(out_ap, in_ap):
            eng = dma_engines[dma_state[0] % len(dma_engines)]
            dma_state[0] += 1
            eng.dma_start(out_ap, in_ap)

        sq_state = [0]

        def squares_and_colsum(src, width, ns_out):
            """ns_out [P, width] (PSUM) <- colsum over partitions of src^2."""
            for h in range(NH):
                xsq = temps.tile([P, HALF, width], bf16, tag="xsq", name="xsq",
                                 bufs=3)
                src_h = src[:, h * HALF:(h + 1) * HALF, :]
                # squares on GPSIMD (2 of 3) and DVE (1 of 3); ACT is the
                # busiest engine so it gets none
                if sq_state[0] % 3 == 2:
                    nc.vector.tensor_mul(xsq[:], src_h, src_h)
                else:
                    nc.gpsimd.tensor_mul(xsq[:], src_h, src_h)
                sq_state[0] += 1
                for k in range(HALF):
                    ko = h * HALF + k
                    nc.tensor.matmul(
                        ns_out,
                        lhsT=ones[:],
                        rhs=xsq[:, k, :],
                        start=(ko == 0),
                        stop=(ko == KO - 1),
                    )

        def rsqrt_act(dst, src_ps):
            """dst <- exp(-0.5*ln(src)); Ln/Exp live in one ACT table set."""
            nc.scalar.activation(dst, src_ps, AF.Ln)
            nc.scalar.activation(dst, dst, AF.Exp, scale=-0.5)

        def normalize(dst, src, rn_ap, width):
            """dst [P, KO, width] (bf16) <- src * rn (rn ko-broadcast)."""
            rn_b = rn_ap[:, None, :].to_broadcast([P, KO, width])
            nc.vector.tensor_mul(dst, src, rn_b)

        # ---- query-side prologue ----
        xqr = temps.tile([P, KO, RPC], f32, tag="xqr", name="xqr", bufs=1)
        dma(xqr[:], xq_r)
        nsq = psum.tile([P, RPC], f32, tag="ps", name="nsq")
        squares_and_colsum(xqr[:], RPC, nsq[:])
        rsqrt_act(rnq[:], nsq[:])
        normalize(xnq_sb[:], xqr[:], rnq[:], RPC)

        # ---- streamed main loop; rn computed per RNC-block chunk ----
        for rc in range(NRN):
            ns_strip = psum.tile([P, RNC, CB], f32, tag="nsb", name="ns_strip",
                                 bufs=1)
            xr_tiles = {}
            for j in range(RNC):
                nb = rc * RNC + j
                cs = slice(nb * CB, (nb + 1) * CB)
                xr = temps.tile([P, KO, CB], f32, tag="xr", name="xr", bufs=4)
                dma(xr[:, 0:HALF, :], x_r[:, 0:HALF, cs])
                dma(xr[:, HALF:KO, :], x_r[:, HALF:KO, cs])
                squares_and_colsum(xr[:], CB, ns_strip[:, j, :])
                xr_tiles[j] = xr  # raw block lives until normalize below
            ccs = slice(rc * RNC * CB, (rc + 1) * RNC * CB)
            rsqrt_act(rn[:, ccs], ns_strip[:].rearrange("p a b -> p (a b)"))
            # normalize + energy for the chunk's blocks
            for j in range(RNC):
                nb = rc * RNC + j
                cs = slice(nb * CB, (nb + 1) * CB)
                normalize(xn_sb[:, :, cs], xr_tiles[j][:], rn[:, cs], CB)
                for m in range(MT):
                    ms = slice(m * P, (m + 1) * P)
                    pt = psum.tile([P, CB], f32, tag="ps", name="pt")
                    for k in range(KO):
                        nc.tensor.matmul(
                            pt[:],
                            lhsT=xnq_sb[:, k, ms],
                            rhs=xn_sb[:, k, cs],
                            start=(k == 0),
                            stop=(k == KO - 1),
                        )
                    nc.scalar.activation(
                        e[:, m, cs], pt[:], AF.Exp,
                        accum_out=sums[:, m, nb:nb + 1],
                    )

        # ---- tail: row-normalize, stream out ----
        OUT_CHUNK = min(N, 2048)
        for m in range(MT):
            nc.vector.tensor_reduce(
                rs[:, m:m + 1], sums[:, m, :], axis=AX.X, op=OP.add
            )
            nc.vector.reciprocal(rr[:, m:m + 1], rs[:, m:m + 1])
            rr_m = rr[:, m:m + 1]
            if m % 2 == 1:
                # materialized scale row for the DVE middle-dim broadcast
                rr_row = temps.tile([P, CB], f32, tag="rr_row", name="rr_row",
                                    bufs=2)
                nc.scalar.activation(rr_row[:], ones_row[:], AF.Copy,
                                     scale=rr_m)
            for ci, c0 in enumerate(range(0, N, OUT_CHUNK)):
                ocs = slice(c0, c0 + OUT_CHUNK)
                if m % 2 == 0:
                    nc.scalar.activation(e[:, m, ocs], e[:, m, ocs], AF.Copy,
                                         scale=rr_m)
                else:
                    ev = e[:, m, ocs].rearrange("p (a b) -> p a b", b=CB)
                    rr_b = rr_row[:, None, :].to_broadcast(
                        [P, OUT_CHUNK // CB, CB])
                    nc.vector.tensor_mul(ev, ev, rr_b)
                dma(out_r[:, m, ocs], e[:, m, ocs])

    nc.compile()
    return nc


def kernel(**inputs) -> np.ndarray:
    global _built, LAST_RESULT
    x = np.ascontiguousarray(np.asarray(inputs["x"], dtype=np.float32))
    C, W, H = x.shape
    N = W * H
    RPC = N // _NCORES
    x2 = x.reshape(C, N)

    if _built is None or _built[1:] != (C, N):
        _built = (_build(C, N, RPC), C, N)
    nc = _built[0]

    from concourse import bass_utils

    in_maps = [
        {"x": x2, "xq": np.ascontiguousarray(x2[:, i * RPC:(i + 1) * RPC])}
        for i in range(_NCORES)
    ]
    kwargs = {}
    if TRACE:
        kwargs["trace"] = True
        if TRACE_CORES is not None:
            kwargs["trace_cores"] = list(TRACE_CORES)
    res = bass_utils.run_bass_kernel_spmd(
        nc, in_maps, core_ids=list(range(_NCORES)), **kwargs
    )
    LAST_RESULT = res
    out = np.concatenate([res.results[i]["out"] for i in range(_NCORES)], axis=0)
    return out.reshape(1, N, N)



# revision 2
# speedup vs baseline: 1.4948x; 1.4948x over previous
"""Cosine-similarity self-attention (Cos_Attn) on 8 Trainium2 NeuronCores.

Reference math (x: [C=512, W=64, H=64] fp32, N = W*H = 4096):
    q = x.reshape(C, N).T                  # [N, C]
    energy = q @ q.T                       # [N, N]
    cos    = energy / (|q_i| |q_j|)
    out    = softmax(cos, axis=-1)[None]   # [1, N, N]

Sharding: N query rows split across 8 cores (512 rows each). One shared
program; per-core asymmetry is handled by ROTATING the input data so that
device-block 0 is always the core's own query block. Host un-rotates the
output columns.

v2 design (per core), all bulk data bf16 (host casts in/out, rel-err
budget 2e-2 >> bf16's 0.4%):
  - input x in block-major dram layout [NB=8, P=128, KO=4, CB=512] bf16
    so each block DMA is 128 x 4KB contiguous descriptors.
  - per block: DVE squares (bf16, 2x/4x mode) -> PE ones-matmul column
    sums (PSUM f32) -> DVE reciprocal -> ACT Sqrt -> rn bf16;
    DVE normalize xn = x * rn (ko-broadcast, validated HW pattern).
  - queries are just xn block 0 (a column slice of the keys) - no
    separate query path at all.
  - energy: 2-block PSUM groups [P, 2, CB], loop k outer / bank inner so
    lhsT stays stationary across banks; 16 groups of 8 matmuls.
  - softmax exp straight out of PSUM on ACT (bf16 out) with accum_out
    row-sums (max-subtraction skipped: cos bounded in [-1, 1]).
  - ACT table discipline (each Sqrt<->Exp switch costs 1.28us; the
    compiler picks tables greedily so interleaving thrashes): batched
    ping-pong [sqrt b0..b3][3 exps][sqrt b4..b7][all remaining exps].
    Copy lives in every table set -> free anywhere.
  - row scale 1/rowsum: ACT Copy materializes rr_row (per-partition
    scale - architectural free-affine operand), DVE bf16 multiply.
  - output streamed per row-tile in 2KB-per-partition chunks, bf16;
    host upcasts to f32.
"""

import numpy as np

_NCORES = 8
_P = 128

# set by the test harness only; the grading path keeps these defaults
TRACE = False
TRACE_CORES = None
LAST_RESULT = None

_built = None  # (nc, C, N)


def _build(C, N):
    from contextlib import ExitStack

    import concourse.tile as tile
    from concourse import bacc, mybir

    f32 = mybir.dt.float32
    bf16 = mybir.dt.bfloat16
    AF = mybir.ActivationFunctionType
    AX = mybir.AxisListType
    OP = mybir.AluOpType

    P = _P
    KO = C // P              # contraction subtiles (4)
    CB = 512                 # column block = one PSUM bank of f32
    NB = N // CB             # 8 column blocks
    MT = (N // _NCORES) // P # 4 query row tiles per core
    GB = 2                   # blocks per energy group (PSUM banks per tile)
    NG = NB // GB            # 4 energy groups per row tile
    SQRT_SPLIT = 4           # sqrt batch 1 = blocks [0, SQRT_SPLIT)
    EARLY_EXPS = 3           # exps between the two sqrt batches

    nc = bacc.Bacc("TRN2", target_bir_lowering=False, debug=False)
    x_d = nc.dram_tensor("x", [NB, P, KO, CB], bf16, kind="ExternalInput")
    out_d = nc.dram_tensor("out", [MT, P, N], bf16, kind="ExternalOutput")

    with tile.TileContext(nc) as tc, ExitStack() as ctx:
        persist = ctx.enter_context(tc.tile_pool(name="persist", bufs=1))
        temps = ctx.enter_context(tc.tile_pool(name="temps", bufs=3))
        psum = ctx.enter_context(tc.tile_pool(name="psum", bufs=2, space="PSUM"))

        xn = persist.tile([P, KO, N], bf16)      # normalized keys (and queries)
        e = persist.tile([P, MT, N], bf16)       # exp(cos); scaled in place
        rn = persist.tile([P, N], bf16)          # 1/|q_j| replicated on parts
        sums = persist.tile([P, MT, NG], f32)    # per-(m, g) exp row sums
        rs = persist.tile([P, MT], f32)
        rr = persist.tile([P, MT], f32)
        ones = persist.tile([P, P], bf16)
        ones_row = persist.tile([P, CB], f32)
        nc.vector.memset(ones[:], 1.0)
        nc.vector.memset(ones_row[:], 1.0)

        xr_tiles = {}

        def dma_in(b):
            xr = temps.tile([P, KO, CB], bf16, tag="xr", name="xr", bufs=4)
            nc.sync.dma_start(xr[:], x_d.ap()[b])
            xr_tiles[b] = xr

        def block_norm_pre(b):
            """squares -> colsum matmul -> reciprocal (no ACT)."""
            xsq = temps.tile([P, KO, CB], bf16, tag="xsq", name="xsq", bufs=2)
            nc.vector.tensor_mul(xsq[:], xr_tiles[b][:], xr_tiles[b][:])
            ns = psum.tile([P, CB], f32, tag="ns", name="ns", bufs=2)
            for k in range(KO):
                nc.tensor.matmul(
                    ns[:], lhsT=ones[:], rhs=xsq[:, k, :],
                    start=(k == 0), stop=(k == KO - 1),
                )
            r1 = temps.tile([P, CB], f32, tag="r1", name="r1", bufs=2)
            nc.vector.reciprocal(r1[:], ns[:])
            return r1

        def block_sqrt(b, r1):
            cs = slice(b * CB, (b + 1) * CB)
            nc.scalar.activation(rn[:, cs], r1[:], AF.Sqrt)

        def block_normalize(b):
            cs = slice(b * CB, (b + 1) * CB)
            rn_b = rn[:, None, cs].to_broadcast([P, KO, CB])
            nc.vector.tensor_mul(xn[:, :, cs], xr_tiles[b][:], rn_b)
            del xr_tiles[b]

        def energy_group(m, g):
            ms = slice(m * P, (m + 1) * P)
            pt = psum.tile([P, GB, CB], f32, tag="pt", name="pt", bufs=3)
            for k in range(KO):
                for j in range(GB):
                    b = g * GB + j
                    cs = slice(b * CB, (b + 1) * CB)
                    nc.tensor.matmul(
                        pt[:, j, :],
                        lhsT=xn[:, k, ms],  # queries = block-0 cols of xn
                        rhs=xn[:, k, cs],
                        start=(k == 0), stop=(k == KO - 1),
                    )
            return pt

        def exp_group(m, g, pt):
            gs = slice(g * GB * CB, (g + 1) * GB * CB)
            nc.scalar.activation(
                e[:, m, gs], pt[:].rearrange("p a b -> p (a b)"), AF.Exp,
                accum_out=sums[:, m, g:g + 1],
            )

        def tail(m, last=False):
            """row scale + output DMA for row tile m."""
            nc.vector.tensor_reduce(
                rs[:, m:m + 1], sums[:, m, :], axis=AX.X, op=OP.add)
            nc.vector.reciprocal(rr[:, m:m + 1], rs[:, m:m + 1])
            rr_row = temps.tile([P, CB], bf16, tag="rr_row", name="rr_row",
                                bufs=2)
            nc.scalar.activation(rr_row[:], ones_row[:], AF.Copy,
                                 scale=rr[:, m:m + 1])
            HC = N // 2
            for h in range(2):
                hs = slice(h * HC, (h + 1) * HC)
                ev = e[:, m, hs].rearrange("p (a b) -> p a b", b=CB)
                rr_b = rr_row[:, None, :].to_broadcast([P, HC // CB, CB])
                nc.vector.tensor_mul(ev, ev, rr_b)
                nc.gpsimd.dma_start(out_d.ap()[m][:, hs], e[:, m, hs])

        # ---- emission: interleaved so each engine queue is in the right
        # order; Tile turns data deps into cross-engine semaphores ----
        for b in range(NB):
            dma_in(b)

        r1s = {}
        for b in range(SQRT_SPLIT):
            r1s[b] = block_norm_pre(b)
            block_sqrt(b, r1s[b])       # ACT batch 1 (sqrt table)
            block_normalize(b)
            del r1s[b]

        # PE: first norm colsums for late blocks happen lazily below; emit
        # their pre-chains now so DVE/PE can run ahead while ACT pingpongs.
        for b in range(SQRT_SPLIT, NB):
            r1s[b] = block_norm_pre(b)

        # energy schedule, g-major; ACT order: 3 early exps, sqrt batch 2,
        # then the rest. Normalize for late blocks emitted right after
        # their sqrt.
        eorder = [(m, g) for g in range(NG) for m in range(MT)]
        pts = {}
        n_emitted = 0
        sqrt2_done = False
        for (m, g) in eorder:
            # PE can only run ~3 groups ahead of exp (PSUM bufs) and needs
            # blocks of group g normalized
            if not sqrt2_done and n_emitted == EARLY_EXPS:
                for b in range(SQRT_SPLIT, NB):
                    block_sqrt(b, r1s[b])   # ACT batch 2 (sqrt table)
                    block_normalize(b)
                    del r1s[b]
                sqrt2_done = True
            pts[(m, g)] = energy_group(m, g)
            exp_group(m, g, pts.pop((m, g)))
            n_emitted += 1
            # emit tails as soon as a row tile's last group is done
            if g == NG - 1:
                tail(m, last=(m == MT - 1))

    nc.compile()
    return nc


def kernel(**inputs) -> np.ndarray:
    global _built, LAST_RESULT
    import ml_dtypes

    x = np.asarray(inputs["x"], dtype=np.float32)
    C, W, H = x.shape
    N = W * H
    P = _P
    KO = C // P
    CB = 512
    NB = N // CB
    MT = (N // _NCORES) // P

    if _built is None or _built[1:] != (C, N):
        _built = (_build(C, N), C, N)
    nc = _built[0]

    from concourse import bass_utils

    # block-major bf16 layout: xin[b, p, ko, c] = x[ko*128+p, b*512+c]
    x2 = x.reshape(KO, P, NB, CB)
    xin = np.ascontiguousarray(
        x2.transpose(2, 1, 0, 3)).astype(ml_dtypes.bfloat16)

    in_maps = [
        {"x": np.ascontiguousarray(np.roll(xin, -c, axis=0))}
        for c in range(_NCORES)
    ]
    kwargs = {}
    if TRACE:
        kwargs["trace"] = True
        if TRACE_CORES is not None:
            kwargs["trace_cores"] = list(TRACE_CORES)
    res = bass_utils.run_bass_kernel_spmd(
        nc, in_maps, core_ids=list(range(_NCORES)), **kwargs
    )
    LAST_RESULT = res
    out = np.empty((N, N), dtype=np.float32)
    for c in range(_NCORES):
        oc = np.asarray(res.results[c]["out"]).astype(np.float32)
        oc = oc.reshape(MT * P, N)          # rows of this core, rotated cols
        out[c * MT * P:(c + 1) * MT * P] = np.roll(oc, c * CB, axis=1)
    return out.reshape(1, N, N)


# revision 5
# speedup vs baseline: 1.9724x; 1.3195x over previous
"""Cosine-similarity self-attention (Cos_Attn) on 8 Trainium2 NeuronCores.

Reference math (x: [C=512, W=64, H=64] fp32, N = W*H = 4096):
    q = x.reshape(C, N).T                  # [N, C]
    energy = q @ q.T                       # [N, N]
    cos    = energy / (|q_i| |q_j|)
    out    = softmax(cos, axis=-1)[None]   # [1, N, N]

Sharding: N query rows split across 8 cores (512 rows each). One shared
program; per-core asymmetry is handled by ROTATING the input data so that
device-block 0 is always the core's own query block. Host un-rotates the
output columns.

v3 design (per core), all bulk data bf16 (host casts in/out; rel-err
budget 2e-2 >> bf16's 0.4%):
  - input x in block-major dram layout [NB=8, P=128, KO=4, CB=512] bf16
    so each block DMA is 128 x 4KB contiguous descriptors.
  - per block: DVE squares (bf16, 2x mode) -> PE ones-matmul column sums
    (PSUM f32) -> DVE reciprocal_approx_fast (1 op; exact reciprocal()
    measured 3.3us/block on HW) -> ACT Sqrt -> rn bf16; DVE normalize
    xn = x * rn (ko-broadcast).
  - queries are xn block 0 (a column slice of the keys) - no separate
    query path.
  - energy: 2-block PSUM groups [P, 2, CB], k outer / bank inner so lhsT
    stays stationary across banks; 16 groups x 8 matmuls.
  - softmax exp straight out of PSUM on ACT (bf16 out) with accum_out
    row sums (max-subtraction skipped: cos bounded in [-1, 1]).
  - ACT table discipline (each Sqrt<->Exp switch costs 1.28us; compiler
    picks tables greedily so interleaving thrashes): batched ping-pong
    [sqrt b0..3][EARLY_EXPS exps][sqrt b4..7][rest]. Copy is in every
    table set -> free anywhere.
  - per-engine queue order is hand-interleaved: PE warmup matmuls during
    the DMA prologue (p-state ramps 0.65->2.4GHz over ~3us of continuous
    work), late-block norm colsums threaded between early energy groups,
    normalize one-behind sqrt on DVE.
  - row scale 1/rowsum: ACT Copy materializes rr_row (per-partition
    scale = free-affine operand), DVE bf16 multiply; output streamed per
    row-tile half, bf16; host upcasts.
"""

import numpy as np

_NCORES = 8
_P = 128

# set by the test harness only; the grading path keeps these defaults
TRACE = False
TRACE_CORES = None
LAST_RESULT = None

_built = None  # (nc, C, N)

# tunables (module-level so a sim harness can sweep them)
GB = 2            # blocks per energy group (PSUM banks per tile)
EARLY_EXPS = 3    # exps between the two sqrt batches
SQRT_SPLIT = 4    # sqrt batch 1 = blocks [0, SQRT_SPLIT)
WARMUP_MM = 6     # junk matmuls to ramp the PE p-state during DMA wait
PT_BUFS = 3


def _build(C, N):
    from contextlib import ExitStack

    import concourse.tile as tile
    from concourse import bacc, mybir

    f32 = mybir.dt.float32
    bf16 = mybir.dt.bfloat16
    AF = mybir.ActivationFunctionType
    AX = mybir.AxisListType
    OP = mybir.AluOpType

    P = _P
    KO = C // P              # contraction subtiles (4)
    CB = 512                 # column block = one PSUM bank of f32
    NB = N // CB             # 8 column blocks
    MT = (N // _NCORES) // P # 4 query row tiles per core
    NG = NB // GB            # energy groups per row tile

    nc = bacc.Bacc("TRN2", target_bir_lowering=False, debug=False)
    x_d = nc.dram_tensor("x", [NB, P, KO, CB], bf16, kind="ExternalInput")
    out_d = nc.dram_tensor("out", [MT, P, N], bf16, kind="ExternalOutput")

    with tile.TileContext(nc) as tc, ExitStack() as ctx:
        persist = ctx.enter_context(tc.tile_pool(name="persist", bufs=1))
        temps = ctx.enter_context(tc.tile_pool(name="temps", bufs=3))
        psum = ctx.enter_context(tc.tile_pool(name="psum", bufs=2, space="PSUM"))

        xn = persist.tile([P, KO, N], bf16)      # normalized keys (and queries)
        e = persist.tile([P, MT, N], bf16)       # exp(cos); scaled in place
        rn = persist.tile([P, N], bf16)          # 1/|q_j| replicated on parts
        sums = persist.tile([P, MT, NG], f32)    # per-(m, g) exp row sums
        rs = persist.tile([P, MT], f32)
        rr = persist.tile([P, MT], f32)
        ones = persist.tile([P, P], bf16)
        ones_row = persist.tile([P, CB], f32)
        nc.vector.memset(ones[:], 1.0)
        nc.vector.memset(ones_row[:], 1.0)

        xr_tiles = {}
        r1s = {}

        def dma_in(b):
            xr = temps.tile([P, KO, CB], bf16, tag="xr", name="xr", bufs=5)
            nc.sync.dma_start(xr[:], x_d.ap()[b])
            xr_tiles[b] = xr

        def warmup_pe():
            junk = psum.tile([P, GB, CB], f32, tag="pt", name="junk",
                             bufs=PT_BUFS)
            for i in range(WARMUP_MM):
                nc.tensor.matmul(junk[:, 0, 0:P], lhsT=ones[:], rhs=ones[:],
                                 start=(i == 0), stop=(i == WARMUP_MM - 1))

        def sq(b):
            xsq = temps.tile([P, KO, CB], bf16, tag="xsq", name="xsq", bufs=2)
            nc.vector.tensor_mul(xsq[:], xr_tiles[b][:], xr_tiles[b][:])
            return xsq

        def nm(b, xsq):
            ns = psum.tile([P, CB], f32, tag="ns", name="ns", bufs=2)
            for k in range(KO):
                nc.tensor.matmul(
                    ns[:], lhsT=ones[:], rhs=xsq[:, k, :],
                    start=(k == 0), stop=(k == KO - 1),
                )
            return ns

        def recip(b, ns):
            r1 = temps.tile([P, CB], f32, tag="r1", name="r1", bufs=2)
            nc.vector.reciprocal_approx_fast(r1[:], ns[:])
            r1s[b] = r1

        def norm_pre(b):
            """squares -> colsum matmul -> approx reciprocal (no ACT)."""
            recip(b, nm(b, sq(b)))

        def block_sqrt(b):
            cs = slice(b * CB, (b + 1) * CB)
            nc.scalar.activation(rn[:, cs], r1s.pop(b)[:], AF.Sqrt)

        def block_normalize(b):
            cs = slice(b * CB, (b + 1) * CB)
            rn_b = rn[:, None, cs].to_broadcast([P, KO, CB])
            nc.vector.tensor_mul(xn[:, :, cs], xr_tiles.pop(b)[:], rn_b)

        def energy_group(m, g):
            ms = slice(m * P, (m + 1) * P)
            pt = psum.tile([P, GB, CB], f32, tag="pt", name="pt", bufs=PT_BUFS)
            for k in range(KO):
                for j in range(GB):
                    b = g * GB + j
                    cs = slice(b * CB, (b + 1) * CB)
                    nc.tensor.matmul(
                        pt[:, j, :],
                        lhsT=xn[:, k, ms],  # queries = block-0 cols of xn
                        rhs=xn[:, k, cs],
                        start=(k == 0), stop=(k == KO - 1),
                    )
            return pt

        def exp_group(m, g, pt):
            gs = slice(g * GB * CB, (g + 1) * GB * CB)
            nc.scalar.activation(
                e[:, m, gs], pt[:].rearrange("p a b -> p (a b)"), AF.Exp,
                accum_out=sums[:, m, g:g + 1],
            )

        def tail(m):
            """row scale + output DMA for row tile m."""
            nc.vector.tensor_reduce(
                rs[:, m:m + 1], sums[:, m, :], axis=AX.X, op=OP.add)
            nc.vector.reciprocal(rr[:, m:m + 1], rs[:, m:m + 1])
            rr_row = temps.tile([P, CB], bf16, tag="rr_row", name="rr_row",
                                bufs=2)
            nc.scalar.activation(rr_row[:], ones_row[:], AF.Copy,
                                 scale=rr[:, m:m + 1])
            HC = N // 2
            for h in range(2):
                hs = slice(h * HC, (h + 1) * HC)
                ev = e[:, m, hs].rearrange("p (a b) -> p a b", b=CB)
                rr_b = rr_row[:, None, :].to_broadcast([P, HC // CB, CB])
                nc.vector.tensor_mul(ev, ev, rr_b)
                nc.gpsimd.dma_start(out_d.ap()[m][:, hs], e[:, m, hs])

        # ---- emission; per-engine queue order is what matters ----
        for b in range(NB):
            dma_in(b)
        warmup_pe()

        # phase A for blocks 0..SQRT_SPLIT-1; normalize one-behind sqrt so
        # DVE doesn't head-block on ACT
        for b in range(SQRT_SPLIT):
            norm_pre(b)
            block_sqrt(b)
            if b > 0:
                block_normalize(b - 1)
        block_normalize(SQRT_SPLIT - 1)

        # energy schedule, g-major, with late-block pre-chains threaded
        # between the first energy groups (PE order: E00 E10 nm4 E20 nm5
        # E30 nm6 E01 nm7 ...)
        eorder = [(m, g) for g in range(NG) for m in range(MT)]
        late = list(range(SQRT_SPLIT, NB))
        n_emitted = 0
        sqrt2_done = False
        for (m, g) in eorder:
            if not sqrt2_done and not late and n_emitted >= EARLY_EXPS:
                for b in range(SQRT_SPLIT, NB):
                    block_sqrt(b)
                    block_normalize(b)
                sqrt2_done = True
            pt = energy_group(m, g)
            exp_group(m, g, pt)
            n_emitted += 1
            if late:
                norm_pre(late.pop(0))
            if g == NG - 1:
                tail(m)

    nc.compile()
    return nc


def kernel(**inputs) -> np.ndarray:
    global _built, LAST_RESULT
    import ml_dtypes

    x = np.asarray(inputs["x"], dtype=np.float32)
    C, W, H = x.shape
    N = W * H
    P = _P
    KO = C // P
    CB = 512
    NB = N // CB
    MT = (N // _NCORES) // P

    if _built is None or _built[1:] != (C, N):
        _built = (_build(C, N), C, N)
    nc = _built[0]

    from concourse import bass_utils

    # block-major bf16 layout: xin[b, p, ko, c] = x[ko*128+p, b*512+c]
    x2 = x.reshape(KO, P, NB, CB)
    xin = np.ascontiguousarray(
        x2.transpose(2, 1, 0, 3)).astype(ml_dtypes.bfloat16)

    in_maps = [
        {"x": np.ascontiguousarray(np.roll(xin, -c, axis=0))}
        for c in range(_NCORES)
    ]
    kwargs = {}
    if TRACE:
        kwargs["trace"] = True
        if TRACE_CORES is not None:
            kwargs["trace_cores"] = list(TRACE_CORES)
    res = bass_utils.run_bass_kernel_spmd(
        nc, in_maps, core_ids=list(range(_NCORES)), **kwargs
    )
    LAST_RESULT = res
    out = np.empty((N, N), dtype=np.float32)
    for c in range(_NCORES):
        oc = np.asarray(res.results[c]["out"]).astype(np.float32)
        oc = oc.reshape(MT * P, N)          # rows of this core, rotated cols
        out[c * MT * P:(c + 1) * MT * P] = np.roll(oc, c * CB, axis=1)
    return out.reshape(1, N, N)
